# revision 1
# baseline (speedup 1.0000x reference)
import numpy as np
import jax
import jax.numpy as jnp
from functools import partial

# nn_LocalWindowMultiHeadAttention: x [1,128,128,128], 8 heads x head_dim 16,
# 7x7 local window (radius 3), reflect padding, Wq/Wk/Wv/Wo 128x128 projections.
# Sharding: split H (128 rows) into 8 shards of 16 rows; each core gets its
# shard plus a 3-row halo on each side taken from the reflect-padded image, so
# no device-to-device exchange is needed.

EMBED_DIM = 128
NUM_HEADS = 8
HEAD_DIM = 16
RADIUS = 3
WIN = 7
H = W = 128
ROWS_PER_CORE = 16
N_CORES = 8


@partial(jax.pmap, in_axes=(0, None, None, None, None, None, None, None, None))
def _shard_attn(xs, Wq, bq, Wk, bk, Wv, bv, Wo, bo):
    # xs: [22, 134, 128] padded slice (16 center rows + 3 halo each side,
    # full padded width).
    scale = 1.0 / np.sqrt(HEAD_DIM)
    Kp = xs @ Wk.T + bk            # [22, 134, C]
    Vp = xs @ Wv.T + bv
    center = xs[RADIUS:RADIUS + ROWS_PER_CORE, RADIUS:RADIUS + W, :]
    q = center @ Wq.T + bq         # [16, 128, C]

    Kw = jnp.stack(
        [Kp[dy:dy + ROWS_PER_CORE, dx:dx + W, :] for dy in range(WIN) for dx in range(WIN)],
        axis=2,
    )                               # [16, 128, 49, C]
    Vw = jnp.stack(
        [Vp[dy:dy + ROWS_PER_CORE, dx:dx + W, :] for dy in range(WIN) for dx in range(WIN)],
        axis=2,
    )

    qh = q.reshape(ROWS_PER_CORE, W, NUM_HEADS, HEAD_DIM)
    Kh = Kw.reshape(ROWS_PER_CORE, W, WIN * WIN, NUM_HEADS, HEAD_DIM)
    Vh = Vw.reshape(ROWS_PER_CORE, W, WIN * WIN, NUM_HEADS, HEAD_DIM)

    scores = jnp.einsum("xyhd,xywhd->xyhw", qh, Kh) * scale
    attn = jax.nn.softmax(scores, axis=-1)
    out = jnp.einsum("xyhw,xywhd->xyhd", attn, Vh).reshape(ROWS_PER_CORE, W, EMBED_DIM)
    return out @ Wo.T + bo          # [16, 128, C]


def kernel(x, Wq, bq, Wk, bk, Wv, bv, Wo, bo):
    x = np.asarray(x, dtype=np.float32)
    xp = np.pad(x, ((0, 0), (RADIUS, RADIUS), (RADIUS, RADIUS), (0, 0)), mode="reflect")[0]
    # [134, 134, 128] -> 8 shards of 22 rows
    shards = np.stack(
        [xp[i * ROWS_PER_CORE: i * ROWS_PER_CORE + ROWS_PER_CORE + 2 * RADIUS] for i in range(N_CORES)]
    )  # [8, 22, 134, 128]
    out = _shard_attn(
        jnp.asarray(shards),
        jnp.asarray(np.asarray(Wq, np.float32)), jnp.asarray(np.asarray(bq, np.float32)),
        jnp.asarray(np.asarray(Wk, np.float32)), jnp.asarray(np.asarray(bk, np.float32)),
        jnp.asarray(np.asarray(Wv, np.float32)), jnp.asarray(np.asarray(bv, np.float32)),
        jnp.asarray(np.asarray(Wo, np.float32)), jnp.asarray(np.asarray(bo, np.float32)),
    )
    out = np.asarray(out)  # [8, 16, 128, 128]
    return out.reshape(1, H, W, EMBED_DIM).astype(np.float32)

